# revision 3
# baseline (speedup 1.0000x reference)
"""Trainium2 Bass kernel for nn_EnsembleTransitionModel.

Sharding: model-parallel (expert-parallel). M=8 ensemble members across 8
NeuronCores; each core runs one full MLP over the whole batch. Inputs are
replicated, per-model weights are sharded.

Device layout: activations are kept feature-major (x^T: [features, batch])
so every weight matrix is used directly as the matmul stationary operand
lhsT=[K_in, M_out] without any transposes. BatchNorm (eval) is folded into
a per-feature affine (scale, bias) applied by one scalar-engine Relu
activation straight out of PSUM. The residual z_last equals rows 1536:1920
of x^T, so the final add reuses x tiles already resident in SBUF.

Matmuls run as float32r (full-rate fp32: 1 PE cycle per moving row for
N>=256 on TRN2) accumulating in fp32 PSUM.
"""

import os
import sys

import numpy as np

for _p in ("/opt/trn_rl_repo", "/root/.axon_site/_ro/trn_rl_repo"):
    if os.path.isdir(_p) and _p not in sys.path:
        sys.path.insert(0, _p)

M = 8
B = 16384
HIST = 5
L = 384
A = 1
HID = 512
NHL = 2
DIN = L * HIST + A * HIST  # 1925
EPS = 1e-5

NCH = 512  # batch columns per chunk (= max fp32 moving dim = 1 PSUM bank)
KT1 = 16  # ceil(DIN / 128); last k-tile has 5 rows
K_LAST = DIN - 15 * 128  # 5
HT = HID // 128  # 4 hidden feature tiles
LT = L // 128  # 3 output feature tiles

# vecs columns: [b1 (4) | s0 (4) | c0 (4) | s1 (4) | c1 (4) | b3 (3)]
COL_B1 = 0
COL_S = lambda l: 4 + 8 * l
COL_C = lambda l: 8 + 8 * l
COL_B3 = 4 + 8 * NHL
NVEC = COL_B3 + LT


def build_bass(batch=B):
    import concourse.bacc as bacc
    import concourse.tile as tile
    from concourse import mybir

    f32 = mybir.dt.float32
    f32r = mybir.dt.float32r
    Relu = mybir.ActivationFunctionType.Relu
    add = mybir.AluOpType.add

    nchunks = batch // NCH
    assert nchunks * NCH == batch

    nc = bacc.Bacc("TRN2", target_bir_lowering=False)
    # float32r: same 32-bit layout (numpy float32), but tagged so the PE
    # runs full-rate fp32 matmuls (1 cycle/row at N>=256 vs 4 for fp32).
    xT = nc.declare_dram_parameter("xT", [DIN, batch], f32r, isOutput=False)
    w1 = nc.declare_dram_parameter("w1", [128, KT1, HID], f32r, isOutput=False)
    wh = nc.declare_dram_parameter("wh", [128, NHL, HT, HT, 128], f32r, isOutput=False)
    w3 = nc.declare_dram_parameter("w3", [128, HT, L], f32r, isOutput=False)
    vecs = nc.declare_dram_parameter("vecs", [128, NVEC], f32, isOutput=False)
    outT = nc.declare_dram_parameter("outT", [L, batch], f32, isOutput=True)

    with tile.TileContext(nc) as tc:
        with (
            tc.tile_pool(name="wt", bufs=1) as wpool,
            tc.tile_pool(name="x", bufs=2) as xpool,
            tc.tile_pool(name="h", bufs=2) as hpool,
            tc.tile_pool(name="o", bufs=2) as opool,
            tc.tile_pool(name="ps", bufs=2, space="PSUM") as pspool,
        ):
            w1_sb = wpool.tile([128, KT1, HID], f32r, tag="w1")
            nc.sync.dma_start(out=w1_sb[:], in_=w1[:])
            wh_sb = wpool.tile([128, NHL, HT, HT, 128], f32r, tag="wh")
            nc.sync.dma_start(out=wh_sb[:], in_=wh[:])
            w3_sb = wpool.tile([128, HT, L], f32r, tag="w3")
            nc.sync.dma_start(out=w3_sb[:], in_=w3[:])
            v_sb = wpool.tile([128, NVEC], f32, tag="vecs")
            nc.sync.dma_start(out=v_sb[:], in_=vecs[:])

            for c in range(nchunks):
                b0 = c * NCH
                # ---- stream x^T chunk: 16 k-tiles of [<=128, NCH] ----
                xts = []
                for kt in range(KT1):
                    rows = 128 if kt < KT1 - 1 else K_LAST
                    xt = xpool.tile([rows, NCH], f32r, tag=f"x{kt}")
                    nc.sync.dma_start(
                        out=xt[:],
                        in_=xT[kt * 128 : kt * 128 + rows, b0 : b0 + NCH],
                    )
                    xts.append(xt)

                # ---- layer 1: h1 = relu(W1^T x + b1), [512, NCH] ----
                h1 = []
                for ht in range(HT):
                    ps = pspool.tile([128, NCH], f32, tag="ps1")
                    for kt in range(KT1):
                        rows = 128 if kt < KT1 - 1 else K_LAST
                        nc.tensor.matmul(
                            ps[:],
                            w1_sb[0:rows, kt, ht * 128 : (ht + 1) * 128],
                            xts[kt][:],
                            start=(kt == 0),
                            stop=(kt == KT1 - 1),
                        )
                    hsb = hpool.tile([128, NCH], f32r, tag=f"h1_{ht}")
                    nc.scalar.activation(
                        hsb[:], ps[:], Relu, bias=v_sb[:, COL_B1 + ht : COL_B1 + ht + 1]
                    )
                    h1.append(hsb)

                # ---- hidden layers: h = relu((h @ Wh[l]) * s_l + c_l) ----
                hin = h1
                for l in range(NHL):
                    hout = []
                    for mt in range(HT):
                        ps = pspool.tile([128, NCH], f32, tag=f"ps2_{l}")
                        for kt in range(HT):
                            nc.tensor.matmul(
                                ps[:],
                                wh_sb[:, l, kt, mt, :],
                                hin[kt][:],
                                start=(kt == 0),
                                stop=(kt == HT - 1),
                            )
                        hsb = hpool.tile([128, NCH], f32r, tag=f"h{l + 2}_{mt}")
                        nc.scalar.activation(
                            hsb[:],
                            ps[:],
                            Relu,
                            bias=v_sb[:, COL_C(l) + mt : COL_C(l) + mt + 1],
                            scale=v_sb[:, COL_S(l) + mt : COL_S(l) + mt + 1],
                        )
                        hout.append(hsb)
                    hin = hout

                # ---- out: delta^T = W3^T h + b3; out = delta^T + zlast^T ----
                # zlast^T is x^T rows 1536:1920 == x k-tiles 12, 13, 14.
                for lt in range(LT):
                    ps = pspool.tile([128, NCH], f32, tag="pso")
                    for kt in range(HT):
                        nc.tensor.matmul(
                            ps[:],
                            w3_sb[:, kt, lt * 128 : (lt + 1) * 128],
                            hin[kt][:],
                            start=(kt == 0),
                            stop=(kt == HT - 1),
                        )
                    ot = opool.tile([128, NCH], f32, tag=f"o{lt}")
                    nc.vector.tensor_tensor(ot[:], ps[:], xts[12 + lt][:].bitcast(f32), add)
                    nc.vector.tensor_scalar_add(
                        ot[:], ot[:], v_sb[:, COL_B3 + lt : COL_B3 + lt + 1]
                    )
                    nc.sync.dma_start(
                        out=outT[lt * 128 : (lt + 1) * 128, b0 : b0 + NCH], in_=ot[:]
                    )
    nc.compile()
    return nc


def prep_core_inputs(z_hist, a_hist, W1, b1, Wh, bh, gamma, beta, rmean, rvar, W3, b3):
    """Host-side shard prep: returns (shared xT, list of per-model dicts)."""
    batch = z_hist.shape[0]
    x = np.concatenate(
        [z_hist.reshape(batch, -1), a_hist.reshape(batch, -1)], axis=1
    ).astype(np.float32)
    xT = np.ascontiguousarray(x.T)  # [DIN, batch]

    rstd = 1.0 / np.sqrt(rvar.astype(np.float64) + EPS)  # [NHL, M, HID]
    s_aff = (gamma * rstd).astype(np.float32)
    c_aff = ((bh - rmean) * gamma * rstd + beta).astype(np.float32)

    in_maps = []
    for m in range(M):
        w1p = np.zeros((KT1 * 128, HID), np.float32)
        w1p[:DIN] = W1[m]
        w1h = np.ascontiguousarray(
            w1p.reshape(KT1, 128, HID).transpose(1, 0, 2)
        )  # [128, KT1, HID]

        whh = np.ascontiguousarray(
            Wh[:, m]  # [NHL, HID, HID]
            .reshape(NHL, HT, 128, HT, 128)
            .transpose(2, 0, 1, 3, 4)
        )  # [128, NHL, kt, mt, 128]

        w3h = np.ascontiguousarray(
            W3[m].reshape(HT, 128, L).transpose(1, 0, 2)
        )  # [128, HT, L]

        vecs = np.zeros((128, NVEC), np.float32)
        vecs[:, COL_B1 : COL_B1 + HT] = b1[m].reshape(HT, 128).T
        for l in range(NHL):
            vecs[:, COL_S(l) : COL_S(l) + HT] = s_aff[l, m].reshape(HT, 128).T
            vecs[:, COL_C(l) : COL_C(l) + HT] = c_aff[l, m].reshape(HT, 128).T
        vecs[:, COL_B3 : COL_B3 + LT] = b3[m].reshape(LT, 128).T

        in_maps.append({"xT": xT, "w1": w1h, "wh": whh, "w3": w3h, "vecs": vecs})
    return in_maps


def kernel(**inputs):
    inputs = {k: np.asarray(v) for k, v in inputs.items()}
    in_maps = prep_core_inputs(**inputs)
    nc = build_bass(B)

    from concourse import bass_utils

    res = bass_utils.run_bass_kernel_spmd(nc, in_maps, core_ids=list(range(M)))
    out = np.stack(
        [np.ascontiguousarray(res.results[m]["outT"].T) for m in range(M)]
    )  # [M, B, L]
    return out.astype(np.float32)
